# revision 3
# baseline (speedup 1.0000x reference)
"""Trainium2 Bass kernel for CriticalBrainDynamics (leaky integrate-and-fire
network with global refractory coupling), SPMD over 8 NeuronCores.

Sharding: data-parallel over batch (128 samples/core); connectivity is
replicated (full [4096,4096] fp8 in SBUF). Device layout is transposed
([neuron, batch]): neuron n = j*128 + p lives at partition p, j-tile j, so
per-neuron params (thresholds, refractory) are [128,32] per-partition
tensors and any(mask, axis=batch) is a free-axis reduction.

With spikes local to each core, the only cross-core dependency is the
refractory update's any-over-full-batch: a [128,32] fp8 (4KB) AllGather per
step + local max over ranks. Collective cost is latency (~5us steady), so:
  - two dummy AllGathers at init absorb the ~50us comm warmup under the
    connectivity load,
  - eligibility is folded into an effective threshold th_eff (= th where
    refractory==0 else +inf), replicated deterministically on all cores;
    when no neuron is globally eligible, a step provably has no spikes and
    every core skips the exchange (tc.If on a replicated register) - the
    refractory countdown steps 2,3 after a global burst cost ~no time.
  - per-core spike mask doubles as the matmul rhs (s^T layout), and the
    v += 0.1*(s@C) matmul is skipped unless this core's batch slice spiked.

All state arithmetic (v, thresholds, refractory) is f32 with the same
per-element operation order as the reference; spikes/connectivity are 0/1
(exact in fp8) accumulated in f32 PSUM, so v evolves bit-identically.
"""

import numpy as np
import ml_dtypes

import concourse.bacc as bacc
import concourse.mybir as mybir
import concourse.tile as tile
from concourse.bass_utils import run_bass_kernel_spmd

N = 4096            # neurons
B = 1024            # batch
N_STEPS = 10
N_CORES = 8
B_LOC = B // N_CORES          # 128 batch samples per core
J = N // 128                  # 32 neuron partition-tiles
KP = J // 2                   # 16 DoubleRow contraction passes
GROUPS = 8                    # psum drain groups (4 out-tiles each)
INF = 1.0e30

F32 = mybir.dt.float32
FP8 = mybir.dt.float8e4
I32 = mybir.dt.int32
AOT = mybir.AluOpType

_CACHE = {}


def build_nc():
    nc = bacc.Bacc("TRN2", target_bir_lowering=False, debug=False,
                   num_devices=N_CORES)

    ext_in = nc.dram_tensor("ext_t", [N, B_LOC], F32, kind="ExternalInput")
    c_in = nc.dram_tensor("c_fp8", [N, N], FP8, kind="ExternalInput")
    mp_in = nc.dram_tensor("mp", [128, J], F32, kind="ExternalInput")
    th_in = nc.dram_tensor("th", [128, J], F32, kind="ExternalInput")
    rf_in = nc.dram_tensor("refr0", [128, J], F32, kind="ExternalInput")
    s_out = nc.dram_tensor("s_out", [N, B_LOC], F32, kind="ExternalOutput")

    with tile.TileContext(nc) as tc:
        with (
            tc.tile_pool(name="sbuf", bufs=1) as pool,
            tc.tile_pool(name="psum", bufs=4, space="PSUM") as pp,
            tc.tile_pool(name="psum2", bufs=2, space="PSUM") as pp2,
            tc.tile_pool(name="dram", bufs=2, space="DRAM") as dp,
        ):
            # --- persistent SBUF state ---
            c_sb = pool.tile([128, J * N], FP8)        # full connectivity
            v = pool.tile([128, J * B_LOC], F32)       # membrane v^T
            mask8 = pool.tile([128, J * B_LOC], FP8)   # spike mask^T (= rhs)
            mask32 = pool.tile([128, J * B_LOC], F32)  # final-step mask f32
            th = pool.tile([128, J], F32)
            th_eff = pool.tile([128, J], F32)          # th, or +inf if refr>0
            refr = pool.tile([128, J], F32)
            counts = pool.tile([128, J], F32)          # max_b v
            gany = pool.tile([128, J], F32)            # global any-spike
            anyloc8 = pool.tile([128, J], FP8)         # local any-spike
            elig8 = pool.tile([128, J], FP8)
            anyv = pool.tile([128, J], I32)
            rgz = pool.tile([128, J], I32)
            three = pool.tile([128, J], F32)
            inf_t = pool.tile([128, J], F32)
            g8 = pool.tile([128, N_CORES * J], FP8)    # gathered flags
            ones8 = pool.tile([128, 1], FP8)
            la = pool.tile([1, 1], F32)
            lai = pool.tile([1, 1], I32)
            ea = pool.tile([1, 1], F32)
            eai = pool.tile([1, 1], I32)
            mp_sb = pool.tile([128, J], F32)
            wz8 = pool.tile([128, J], FP8)

            c3 = c_sb[:].rearrange("p (k m) -> p k m", k=J)
            m3 = mask8[:].rearrange("p (k b) -> p k b", k=J)
            v3 = v[:].rearrange("p (j b) -> p j b", j=J)

            def th_eff_bcast():
                return th_eff[:].unsqueeze(-1).broadcast_to([128, J, B_LOC])

            # --- collective warmup: 2 dummy AllGathers, overlapped with the
            # connectivity load (first collective pays ~50us comm bringup).
            nc.gpsimd.memset(wz8[:], 0.0)
            for _ in range(2):
                win = dp.tile([128, J], FP8, tag="agin")
                wout = dp.tile([128 * N_CORES, J], FP8, addr_space="Shared",
                               tag="agout")
                nc.sync.dma_start(win[:], wz8[:])
                nc.gpsimd.collective_compute(
                    "AllGather", AOT.bypass,
                    ins=[win[:].opt()], outs=[wout[:].opt()],
                    replica_groups=[list(range(N_CORES))])

            # --- loads: connectivity split over 2 HWDGE queues, k-ascending
            for q in range(4):
                eng = nc.sync if q % 2 == 0 else nc.scalar
                ks = slice(q * (J // 4), (q + 1) * (J // 4))
                eng.dma_start(
                    c3[:, ks, :],
                    c_in.ap().rearrange("(k p) m -> p k m", p=128)[:, ks, :])
            nc.gpsimd.dma_start(v3, ext_in.ap().rearrange(
                "(j p) b -> p j b", p=128))
            nc.sync.dma_start(th[:], th_in.ap())
            nc.sync.dma_start(refr[:], rf_in.ap())
            nc.sync.dma_start(mp_sb[:], mp_in.ap())
            nc.gpsimd.memset(three[:], 3.0)
            nc.gpsimd.memset(inf_t[:], INF)
            nc.gpsimd.memset(ones8[:], 1.0)
            nc.gpsimd.memset(mask32[:], 0.0)
            nc.gpsimd.memset(anyloc8[:], 0.0)

            # v0 = ext + membrane_potentials (per-partition add per j-tile)
            for j in range(J):
                nc.vector.tensor_scalar_add(
                    v[:, j * B_LOC:(j + 1) * B_LOC],
                    v[:, j * B_LOC:(j + 1) * B_LOC], mp_sb[:, j:j + 1])

            # th_eff = th where refr==0 else +inf
            nc.vector.tensor_copy(th_eff[:], th[:])
            nc.vector.tensor_scalar(
                out=rgz[:], in0=refr[:], scalar1=0.0, scalar2=None,
                op0=AOT.is_gt)
            nc.vector.copy_predicated(th_eff[:], rgz[:], inf_t[:])

            # r_elig for step 1: any neuron with refr==0 anywhere
            nc.vector.tensor_scalar(
                out=elig8[:], in0=refr[:], scalar1=0.0, scalar2=None,
                op0=AOT.is_equal)
            el_ps = pp2.tile([1, J], F32, tag="elps")
            nc.tensor.matmul(el_ps[:], ones8[:], elig8[:], start=True,
                             stop=True)
            nc.vector.tensor_reduce(out=ea[:], in_=el_ps[:],
                                    axis=mybir.AxisListType.X, op=AOT.max)
            nc.vector.tensor_copy(eai[:], ea[:])
            eregs = nc.alloc_registers("eligreg0")
            nc.regs_load(eregs, eai[0:1, 0:1])
            elig_sv = nc.snap(eregs, donate=True)

            loc_sv = None  # local any-spike of previous step

            for step in range(1, N_STEPS + 1):
                last = step == N_STEPS

                # --- network input: v += 0.1 * (s_prev @ C), local batch ---
                if step >= 2:
                    with tc.If(loc_sv > 0):
                        for g in range(GROUPS):
                            ps = pp.tile([128, 4 * B_LOC], F32, tag="ps")
                            for tt in range(4):
                                t = 4 * g + tt
                                pcol = ps[:, tt * B_LOC:(tt + 1) * B_LOC]
                                for kp in range(KP):
                                    nc.tensor.matmul(
                                        pcol,
                                        c3[:, 2 * kp:2 * kp + 2,
                                           t * 128:(t + 1) * 128],
                                        m3[:, 2 * kp:2 * kp + 2, :],
                                        start=(kp == 0),
                                        stop=(kp == KP - 1),
                                        perf_mode=mybir.MatmulPerfMode.DoubleRow,
                                    )
                            vs = v[:, g * 4 * B_LOC:(g + 1) * 4 * B_LOC]
                            nc.vector.scalar_tensor_tensor(
                                out=vs, in0=ps[:], scalar=0.1, in1=vs,
                                op0=AOT.mult, op1=AOT.add)

                if last:
                    # output = spikes of step 10; th_eff already encodes
                    # eligibility so this is correct even when all-refractory
                    nc.vector.tensor_tensor(
                        out=mask32[:].rearrange("p (j b) -> p j b", j=J),
                        in0=v3, in1=th_eff_bcast(), op=AOT.is_gt)
                    nc.sync.dma_start(
                        s_out.ap().rearrange("(j p) b -> p j b", p=128),
                        mask32[:].rearrange("p (j b) -> p j b", j=J))
                    break

                # anyloc8 must be 0 when the eligible block is skipped
                if step > 1:
                    nc.gpsimd.memset(anyloc8[:], 0.0)

                with tc.If(elig_sv > 0):
                    # counts[p,j] = max_b v ; local any = counts > th_eff
                    nc.vector.tensor_reduce(
                        out=counts[:], in_=v3,
                        axis=mybir.AxisListType.X, op=AOT.max)
                    nc.vector.tensor_tensor(
                        out=anyloc8[:], in0=counts[:], in1=th_eff[:],
                        op=AOT.is_gt)
                    # 4KB flag AllGather + local max over ranks
                    ag_in = dp.tile([128, J], FP8, tag="agin")
                    ag_out = dp.tile([128 * N_CORES, J], FP8,
                                     addr_space="Shared", tag="agout")
                    nc.sync.dma_start(ag_in[:], anyloc8[:])
                    nc.gpsimd.collective_compute(
                        "AllGather", AOT.bypass,
                        ins=[ag_in[:].opt()], outs=[ag_out[:].opt()],
                        replica_groups=[list(range(N_CORES))])
                    nc.scalar.dma_start(
                        g8[:].rearrange("p (r j) -> p r j", r=N_CORES),
                        ag_out[:].rearrange("(r p) j -> p r j", p=128))
                    nc.vector.tensor_reduce(
                        out=gany[:],
                        in_=g8[:].rearrange("p (r j) -> p j r", r=N_CORES),
                        axis=mybir.AxisListType.X, op=AOT.max)
                    # refractory set (decrement happens unconditionally below)
                    nc.vector.tensor_scalar(
                        out=anyv[:], in0=gany[:], scalar1=0.0, scalar2=None,
                        op0=AOT.is_gt)
                    nc.vector.copy_predicated(refr[:], anyv[:], three[:])

                # local-any register (0 when skipped: anyloc8 was memset)
                la_ps = pp2.tile([1, J], F32, tag="laps")
                nc.tensor.matmul(la_ps[:], ones8[:], anyloc8[:], start=True,
                                 stop=True)
                nc.vector.tensor_reduce(out=la[:], in_=la_ps[:],
                                        axis=mybir.AxisListType.X, op=AOT.max)
                nc.vector.tensor_copy(lai[:], la[:])
                lregs = nc.alloc_registers(f"locreg{step}")
                nc.regs_load(lregs, lai[0:1, 0:1])
                loc_sv = nc.snap(lregs, donate=True)

                with tc.If(loc_sv > 0):
                    # spike mask (rhs for next matmul) + membrane reset
                    nc.vector.tensor_tensor(
                        out=m3, in0=v3, in1=th_eff_bcast(), op=AOT.is_gt)
                    nc.vector.scalar_tensor_tensor(
                        out=v[:], in0=mask8[:], scalar=0.0, in1=v[:],
                        op0=AOT.is_equal, op1=AOT.mult)

                # refractory decrement + th_eff rebuild (replicated state)
                nc.vector.tensor_scalar(
                    out=refr[:], in0=refr[:], scalar1=1.0, scalar2=0.0,
                    op0=AOT.subtract, op1=AOT.max)
                nc.vector.tensor_copy(th_eff[:], th[:])
                nc.vector.tensor_scalar(
                    out=rgz[:], in0=refr[:], scalar1=0.0, scalar2=None,
                    op0=AOT.is_gt)
                nc.vector.copy_predicated(th_eff[:], rgz[:], inf_t[:])

                # r_elig for next step
                nc.vector.tensor_scalar(
                    out=elig8[:], in0=refr[:], scalar1=0.0, scalar2=None,
                    op0=AOT.is_equal)
                el_ps = pp2.tile([1, J], F32, tag="elps")
                nc.tensor.matmul(el_ps[:], ones8[:], elig8[:], start=True,
                                 stop=True)
                nc.vector.tensor_reduce(out=ea[:], in_=el_ps[:],
                                        axis=mybir.AxisListType.X, op=AOT.max)
                nc.vector.tensor_copy(eai[:], ea[:])
                eregs = nc.alloc_registers(f"eligreg{step}")
                nc.regs_load(eregs, eai[0:1, 0:1])
                elig_sv = nc.snap(eregs, donate=True)

                # leak
                nc.scalar.mul(v[:], v[:], 0.95)

    nc.compile()
    return nc


def _prep_inputs(external_input, connectivity, membrane_potentials,
                 thresholds, refractory_periods):
    """Shard + lay out the full inputs for the 8 per-core NEFF input maps."""
    ext = np.ascontiguousarray(external_input, dtype=np.float32)
    conn = np.ascontiguousarray(connectivity, dtype=np.float32)
    mp = np.asarray(membrane_potentials, dtype=np.float32)
    th = np.asarray(thresholds, dtype=np.float32)
    rf = np.asarray(refractory_periods, dtype=np.float32)

    c_fp8 = conn.astype(ml_dtypes.float8_e4m3)               # [4096, 4096]

    # [4096] -> [128, 32] with n = j*128 + p  ->  arr[p, j]
    def vec_tile(x):
        return np.ascontiguousarray(x.reshape(J, 128).T)

    mp_t, th_t, rf_t = vec_tile(mp), vec_tile(th), vec_tile(rf)

    in_maps = []
    for c in range(N_CORES):
        sl = slice(c * B_LOC, (c + 1) * B_LOC)
        ext_t = np.ascontiguousarray(ext[sl, :].T)           # [4096, 128]
        in_maps.append({
            "ext_t": ext_t,
            "c_fp8": c_fp8,
            "mp": mp_t,
            "th": th_t,
            "refr0": rf_t,
        })
    return in_maps


def kernel(external_input, connectivity, membrane_potentials, thresholds,
           refractory_periods, _trace=False):
    if "nc" not in _CACHE:
        _CACHE["nc"] = build_nc()
    nc = _CACHE["nc"]
    in_maps = _prep_inputs(external_input, connectivity, membrane_potentials,
                           thresholds, refractory_periods)
    res = run_bass_kernel_spmd(nc, in_maps, core_ids=list(range(N_CORES)),
                               trace=_trace)
    _CACHE["last_results"] = res
    out = np.empty((B, N), dtype=np.float32)
    for c in range(N_CORES):
        out[c * B_LOC:(c + 1) * B_LOC, :] = res.results[c]["s_out"].T
    return out
